# revision 17
# baseline (speedup 1.0000x reference)
"""Trainium2 Bass kernel for nn_MessageBlock (GNN message passing).

Sharding: edges sorted by destination node, sharded across 8 cores by
destination range (2500 nodes each) -- every core owns its output slice,
so no inter-core collective is needed.  Per core, edges are bucketed into
20 tiles of 128 destination nodes; each 128-edge chunk's messages are
scatter-added into a PSUM accumulator with a one-hot matmul on the tensor
engine (exact for duplicate destinations).  Source-node features (the
s_pass MLP table and v) are fetched per edge with gpsimd dma_gather.

Device pipeline per core:
  phase 0: s_pass = silu(s@W1+b1)@W2+b2 for all 20096 (padded) nodes,
           written to a bf16 DRAM table.  Transpose-free: W1 is the
           stationary operand so the product comes out feature-major.
  phase 1: per node tile: gather s_pass/v rows by source index, build
           RBF features with ACT sin + an f32r matmul, form messages in
           bf16 on DVE/GPSIMD, scatter via one-hot matmuls into PSUM,
           add the s/v base values, DMA out.
"""
import sys
import dataclasses

sys.path.insert(0, "/opt/trn_rl_repo")

import numpy as np
import ml_dtypes

import concourse.bacc as bacc
import concourse.bass as bass
import concourse.mybir as mybir
from concourse.bass_utils import run_bass_kernel_spmd
from concourse.library_config import mlp

F32 = mybir.dt.float32
F32R = mybir.dt.float32r
BF16 = mybir.dt.bfloat16
I16 = mybir.dt.int16
MULT = mybir.AluOpType.mult
ADD = mybir.AluOpType.add
AF = mybir.ActivationFunctionType

N_NODES = 20000
EMB = 128
N_RBF = 20
R_CUT = 5.0
NCORES = 8
NPC = N_NODES // NCORES          # 2500 nodes per core
TILES = (NPC + 127) // 128       # 20 node tiles per core
NT0 = (N_NODES + 127) // 128     # 157 s_pass tiles
NPAD = NT0 * 128                 # 20096
QUADS0 = (NT0 + 3) // 4          # 40
NPADQ = QUADS0 * 512             # 20480
RQ = (TILES + 2) // 3            # 7 rbf groups (3 tiles each; base 96 illegal)

_BF = ml_dtypes.bfloat16


def _bcast_last(ap, n):
    """View an AP whose last dim is 1 as broadcast to n (stride 0)."""
    new = list(ap.ap)
    new[-1] = (0, n)
    return dataclasses.replace(ap, ap=new)


def _build(C, stage=3):
    """Build the SPMD program; C = edge chunks per node tile (multiple of 6)."""
    EC = C * 128
    H = C // 2
    nc = bacc.Bacc("TRN2")

    d_sT = nc.declare_dram_parameter("sT", [128, NPADQ], F32R, isOutput=False)
    d_W1 = nc.declare_dram_parameter("W1", [128, EMB], F32R, isOutput=False)
    d_W2 = nc.declare_dram_parameter("W2", [128, 3 * EMB], F32R, isOutput=False)
    d_Waug = nc.declare_dram_parameter("Waug", [128, 3 * EMB], F32R, isOutput=False)
    d_b1 = nc.declare_dram_parameter("b1c", [128, 1], F32, isOutput=False)
    d_b2 = nc.declare_dram_parameter("b2r", [1, 3 * EMB], F32R, isOutput=False)
    d_ones = nc.declare_dram_parameter("ones", [1, 128], F32R, isOutput=False)
    d_vtab = nc.declare_dram_parameter("vtab", [NPAD, 3 * EMB], BF16, isOutput=False)
    d_srci = nc.declare_dram_parameter("srci", [TILES, 128, C * 8], I16, isOutput=False)
    d_rn3 = nc.declare_dram_parameter("rn3", [128, TILES * C, 3], F32, isOutput=False)
    d_ohd = nc.declare_dram_parameter("ohd", [TILES, 128, EC], BF16, isOutput=False)
    d_argA = nc.declare_dram_parameter("argA", [RQ, 128, EC], F32, isOutput=False)
    d_argG = nc.declare_dram_parameter("argG", [RQ, 128, EC], F32, isOutput=False)
    d_ribc = nc.declare_dram_parameter("ribc", [RQ, 128, EC], F32, isOutput=False)
    d_base = nc.declare_dram_parameter("base", [TILES, 128, 512], F32, isOutput=False)
    d_out = nc.declare_dram_parameter("out", [TILES, 128, 512], F32, isOutput=True)
    d_spass = nc.dram_tensor("spass", [NPAD, 3 * EMB], BF16)

    ctx = []

    def sb(name, shape, dtype):
        t = nc.sbuf_tensor(name, shape, dtype)
        ctx.append(t)
        return t.__enter__()

    def ps(name, shape):
        t = nc.psum_tensor(name, shape, F32)
        ctx.append(t)
        return t.__enter__()

    def sem(name):
        t = nc.semaphore(name)
        ctx.append(t)
        return t.__enter__()

    # SBUF
    b_W1 = sb("b_W1", [128, EMB], F32R)
    b_W2 = sb("b_W2", [128, 3 * EMB], F32R)
    b_Waug = sb("b_Waug", [128, 3 * EMB], F32R)
    b_b1 = sb("b_b1", [128, 1], F32)
    b_b2 = sb("b_b2", [1, 3 * EMB], F32R)
    b_ones = sb("b_ones", [1, 128], F32R)
    b_rn3 = sb("b_rn3", [128, TILES * C, 3], F32)
    b_sT = [sb(f"b_sT{i}", [128, 512], F32R) for i in range(2)]
    b_hs = [sb(f"b_hs{i}", [128, 512], F32R) for i in range(2)]
    b_sp = [sb(f"b_sp{i}", [128, 3 * EMB], BF16) for i in range(4)]
    b_argA = sb("b_argA", [128, EC], F32)
    b_argG = sb("b_argG", [128, EC], F32)
    b_ribc = sb("b_ribc", [128, EC], F32)
    b_sin = sb("b_sin", [128, EC], F32)
    b_rbf = sb("b_rbf", [128, EC], F32R)
    b_srci = [sb(f"b_srci{i}", [128, C * 8], I16) for i in range(2)]
    b_sg = [sb(f"b_sg{i}", [128, C, 384], BF16) for i in range(2)]
    b_vg = [sb(f"b_vg{i}", [128, C, 384], BF16) for i in range(2)]
    b_oh = [sb(f"b_oh{i}", [128, EC], BF16) for i in range(2)]
    b_base = [sb(f"b_base{i}", [128, 512], F32) for i in range(2)]
    b_out = [sb(f"b_out{i}", [128, 512], F32) for i in range(2)]
    # half-tile buffers; msg cols: [ds 0:128 | vgate 128:512 | rep 512:640 | rhs_d 640:1024]
    b_sq = [sb(f"b_sq{i}", [128, H, 384], BF16) for i in range(3)]
    b_msg = [sb(f"b_msg{i}", [128, H, 1024], BF16) for i in range(2)]
    b_gate = [sb(f"b_gate{i}", [128, H, 128], BF16) for i in range(2)]

    # PSUM (8 banks)
    p_out = [ps(f"p_out{i}", [128, 512]) for i in range(2)]
    p_t = ps("p_t", [128, 3 * 512])
    p_h = ps("p_h", [128, 512])
    p_sp = [ps(f"p_sp{i}", [128, 512]) for i in range(2)]

    s_c0 = sem("s_c0")
    s_sTin = sem("s_sTin")
    s_h = sem("s_h")
    s_silu = sem("s_silu")
    s_mm2 = sem("s_mm2")
    s_evD = sem("s_evD")     # phase-0 evac, even tiles (DVE)
    s_evA = sem("s_evA")     # phase-0 evac, odd tiles (ACT)
    s_spw = sem("s_spw")
    s_rbin = sem("s_rbin")
    s_sin = sem("s_sin")
    s_rbf = sem("s_rbf")
    s_ein = sem("s_ein")
    s_gat = sem("s_gat")
    s_tmm = sem("s_tmm")
    s_sq = sem("s_sq")
    s_msg = sem("s_msg")     # DVE per-half msg ops (before rhs_d d=1,2)
    s_rhsd = sem("s_rhsd")   # gpsimd rhs_d per half
    s_smmh = sem("s_smmh")   # PE scatter per half
    s_acc = sem("s_acc")
    s_outw = sem("s_outw")

    NCONST = 7  # W1 W2 Waug b1 b2 ones rn3

    with nc.Block() as block:
        # ---------------- SYNC: all bulk DMA ----------------
        @block.sync
        def _(e):
            e.dma_start(out=b_W1[:], in_=d_W1[:]).then_inc(s_c0, 16)
            e.dma_start(out=b_W2[:], in_=d_W2[:]).then_inc(s_c0, 16)
            e.dma_start(out=b_Waug[:], in_=d_Waug[:]).then_inc(s_c0, 16)
            e.dma_start(out=b_b1[:], in_=d_b1[:]).then_inc(s_c0, 16)
            e.dma_start(out=b_b2[:], in_=d_b2[:]).then_inc(s_c0, 16)
            e.dma_start(out=b_ones[:], in_=d_ones[:]).then_inc(s_c0, 16)
            e.dma_start(out=b_rn3[:], in_=d_rn3[:]).then_inc(s_c0, 16)
            def spass_store_quad(qq):
                for t in range(4 * qq, min(4 * qq + 4, NT0)):
                    e.wait_ge(s_evD if t % 2 == 0 else s_evA, t // 2 + 1)
                    e.dma_start(out=d_spass[t * 128:(t + 1) * 128, :],
                                in_=b_sp[t % 4][:]).then_inc(s_spw, 16)

            for q in range(QUADS0):
                if q >= 2:
                    e.wait_ge(s_h, q - 1)
                e.dma_start(out=b_sT[q % 2][:],
                            in_=d_sT[:, q * 512:(q + 1) * 512]).then_inc(s_sTin, 16)
                if q >= 2:
                    spass_store_quad(q - 2)
            spass_store_quad(QUADS0 - 2)
            spass_store_quad(QUADS0 - 1)
            if stage == 1:
                # debug: dump spass rows into out
                for t in range(TILES):
                    e.wait_ge(s_spw, 16 * NT0)
                    e.dma_start(out=d_out[t][:].bitcast(BF16)[:, 0:384],
                                in_=d_spass[t * 128:(t + 1) * 128, :]).then_inc(s_outw, 16)
                e.wait_ge(s_outw, 16 * TILES)
                e.wait_ge(s_spw, 16 * NT0)
                return
            if stage == 2:
                # srci loads + dump gathered sg chunk0 per tile
                for t in range(TILES):
                    if t >= 2:
                        e.wait_ge(s_gat, 96 * (t - 1))
                    e.dma_start(out=b_srci[t % 2][:], in_=d_srci[t]).then_inc(s_ein, 16)
                    e.wait_ge(s_gat, 96 * (t + 1))
                    e.dma_start(out=d_out[t][:].bitcast(BF16)[:, 0:768],
                                in_=b_sg[t % 2][:, 0:2, :]).then_inc(s_outw, 16)
                    e.dma_start(out=d_out[t][:].bitcast(BF16)[:, 768:1024][:, 0:256],
                                in_=b_vg[t % 2][:, 0, 0:256]).then_inc(s_outw, 16)
                e.wait_ge(s_outw, 32 * TILES)
                e.wait_ge(s_spw, 16 * NT0)
                return
            for t in range(TILES):
                q = t // 3
                if t % 3 == 0:
                    if q >= 1:
                        e.wait_ge(s_tmm, 3 * q * C)
                    e.dma_start(out=b_argA[:], in_=d_argA[q]).then_inc(s_rbin, 16)
                    e.dma_start(out=b_argG[:], in_=d_argG[q]).then_inc(s_rbin, 16)
                    e.dma_start(out=b_ribc[:], in_=d_ribc[q]).then_inc(s_rbin, 16)
                if t >= 2:
                    e.wait_ge(s_gat, 96 * (t - 1))   # srci consumed by gather t-2
                    e.wait_ge(s_smmh, 2 * (t - 1))   # oh consumed by scatter t-2
                    e.wait_ge(s_acc, t - 1)          # base consumed by evac t-2
                e.dma_start(out=b_srci[t % 2][:], in_=d_srci[t]).then_inc(s_ein, 16)
                e.dma_start(out=b_oh[t % 2][:], in_=d_ohd[t]).then_inc(s_ein, 16)
                e.dma_start(out=b_base[t % 2][:], in_=d_base[t]).then_inc(s_ein, 16)
                if t >= 2:
                    e.wait_ge(s_acc, t - 1)
                    e.dma_start(out=d_out[t - 2],
                                in_=b_out[t % 2][:]).then_inc(s_outw, 16)
            for t in range(TILES - 2, TILES):
                e.wait_ge(s_acc, t + 1)
                e.dma_start(out=d_out[t], in_=b_out[t % 2][:]).then_inc(s_outw, 16)
            e.wait_ge(s_outw, 16 * TILES)
            e.wait_ge(s_spw, 16 * NT0)

        # ---------------- PE ----------------
        @block.tensor
        def _(e):
            e.wait_ge(s_c0, 16 * NCONST)
            w1r = b_W1[:]
            w2r = b_W2[:]
            onesr = b_ones[:]
            b2r = b_b2[:]

            def mm2_quad(q):
                for u in range(4):
                    t = q * 4 + u
                    if t >= NT0:
                        return
                    if t >= 2:
                        ev = s_evD if (t - 2) % 2 == 0 else s_evA
                        e.wait_ge(ev, (t - 2) // 2 + 1)
                    hsr = b_hs[q % 2][:, u * 128:(u + 1) * 128]
                    e.matmul(p_sp[t % 2][:, 0:384], hsr, w2r,
                             start=True, stop=False, skip_group_check=True)
                    e.matmul(p_sp[t % 2][:, 0:384], onesr, b2r,
                             start=False, stop=True,
                             skip_group_check=True).then_inc(s_mm2, 1)

            for q in range(QUADS0):
                e.wait_ge(s_sTin, 16 * (q + 1))
                if q >= 1:
                    e.wait_ge(s_silu, q)      # p_h consumed by silu(q-1)
                e.matmul(p_h[:], w1r, b_sT[q % 2][:],
                         start=True, stop=True, skip_group_check=True).then_inc(s_h, 1)
                if q >= 1:
                    e.wait_ge(s_silu, q)
                    mm2_quad(q - 1)
            e.wait_ge(s_silu, QUADS0)
            mm2_quad(QUADS0 - 1)

            def scatter(t):
                ohb = b_oh[t % 2]
                if t >= 2:
                    e.wait_ge(s_acc, t - 1)   # p_out[t%2] free
                for hh in range(2):
                    ht = 2 * t + hh
                    e.wait_ge(s_msg, ht + 1)
                    e.wait_ge(s_rhsd, ht + 1)
                    mb = b_msg[ht % 2]
                    last = None
                    for c in range(H):
                        cg = hh * H + c
                        lhs = ohb[:, cg * 128:(cg + 1) * 128]
                        e.matmul(p_out[t % 2][:], lhs, mb[:, c, 0:512],
                                 start=(cg == 0), stop=False, skip_group_check=True)
                        for d in range(3):
                            last = e.matmul(
                                p_out[t % 2][:, 128 + d * 128:256 + d * 128],
                                lhs, mb[:, c, 640 + d * 128:768 + d * 128],
                                start=False, stop=(cg == C - 1 and d == 2),
                                skip_group_check=True)
                    last.then_inc(s_smmh, 1)

            if stage < 3:
                return
            # phase 1: t-matmuls for tile t, then scatter for tile t-1
            for t in range(TILES):
                q = t // 3
                e.wait_ge(s_rbf, q + 1)
                e.wait_ge(s_ein, 16 * (3 * t + 2))   # oh(t) loaded (for scatter later)
                for c in range(C):
                    k = t * C + c
                    if k >= 3:
                        e.wait_ge(s_sq, (k - 3) // 3 + 1)  # p_t bank free
                    lhs = b_rbf[32 * (t % 3):32 * (t % 3) + 21,
                                c * 128:(c + 1) * 128]
                    rhs = b_Waug[32 * (t % 3):32 * (t % 3) + 21, :]
                    e.matmul(p_t[:, (c % 3) * 512:(c % 3) * 512 + 384],
                             lhs, rhs, start=True, stop=True,
                             skip_group_check=True).then_inc(s_tmm, 1)
                if t >= 1:
                    scatter(t - 1)
            scatter(TILES - 1)

        # ---------------- ACT ----------------
        @block.scalar
        def _(e):
            e.wait_ge(s_c0, 16 * NCONST)

            def evac_odd(qq):
                for t in range(4 * qq, min(4 * qq + 4, NT0)):
                    if t % 2 != 1:
                        continue
                    e.wait_ge(s_mm2, t + 1)
                    if t >= 4:
                        e.wait_ge(s_spw, 16 * (t - 3))
                    e.activation(b_sp[t % 4][:], p_sp[t % 2][:, 0:384],
                                 AF.Copy).then_inc(s_evA, 1)

            for q in range(QUADS0):
                if q >= 2:
                    evac_odd(q - 2)     # before silu(q): breaks mm2->evac cycle
                e.wait_ge(s_h, q + 1)
                if q >= 2:
                    lim = min(4 * (q - 1), NT0)
                    e.wait_ge(s_mm2, lim)    # b_hs[q%2] consumed
                e.activation(b_hs[q % 2][:], p_h[:], AF.Silu,
                             bias=b_b1[:, 0:1], scale=1.0).then_inc(s_silu, 1)
            evac_odd(QUADS0 - 2)
            evac_odd(QUADS0 - 1)
            if stage < 3:
                return
            # phase 1
            for t in range(TILES):
                q = t // 3
                if t % 3 == 0:
                    e.wait_ge(s_rbin, 48 * (q + 1))
                    e.activation(b_sin[:], b_argA[:], AF.Sin).then_inc(s_sin, 1)
                    e.activation(b_argA[:], b_argG[:], AF.Sin).then_inc(s_sin, 1)
                for bb in range(C // 3):
                    k0 = t * C + bb * 3
                    e.wait_ge(s_tmm, k0 + 3)
                    hh = (bb * 3) // H
                    lc = (bb * 3) % H
                    ht = 2 * t + hh
                    if ht >= 3:
                        e.wait_ge(s_msg, ht - 2)   # b_sq[ht%3] consumed
                    src = dataclasses.replace(
                        p_t[:], ap=[p_t[:].ap[0], (512, 3), (1, 384)])
                    e.activation(b_sq[ht % 3][:, lc:lc + 3, :], src,
                                 AF.Square).then_inc(s_sq, 1)

        # ---------------- DVE ----------------
        @block.vector
        def _(e):
            e.wait_ge(s_c0, 16 * NCONST)
            # phase-0 evac, even tiles
            for t in range(0, NT0, 2):
                e.wait_ge(s_mm2, t + 1)
                if t >= 4:
                    e.wait_ge(s_spw, 16 * (t - 3))
                e.tensor_copy(b_sp[t % 4][:],
                              p_sp[t % 2][:, 0:384]).then_inc(s_evD, 1)

            if stage < 3:
                return

            def evac(t):
                e.wait_ge(s_smmh, 2 * (t + 1))
                e.wait_ge(s_ein, 16 * (3 * t + 3))
                if t >= 2:
                    e.wait_ge(s_outw, 16 * (t - 1))
                e.tensor_tensor(b_out[t % 2][:], p_out[t % 2][:],
                                b_base[t % 2][:], ADD).then_inc(s_acc, 1)

            for t in range(TILES):
                q = t // 3
                if t % 3 == 0:
                    e.wait_ge(s_sin, 2 * (q + 1))
                    e.tensor_tensor(b_argA[:], b_argA[:], b_ribc[:], MULT)
                    e.tensor_tensor(b_rbf[:], b_sin[:], b_argA[:],
                                    MULT).then_inc(s_rbf, 1)
                for hh in range(2):
                    ht = 2 * t + hh
                    e.wait_ge(s_gat, 96 * t + (64 if hh == 0 else 96))
                    sqb = b_sq[ht % 3]
                    mgb = b_msg[ht % 2]
                    gtb = b_gate[ht % 2]
                    sgb = b_sg[t % 2]
                    vgb = b_vg[t % 2]
                    co = hh * H
                    e.wait_ge(s_sq, t * (C // 3) + (hh + 1) * (H // 3))
                    if ht >= 2:
                        e.wait_ge(s_smmh, ht - 1)  # msg/gate buf consumed
                    e.tensor_tensor(mgb[:, :, 0:128], sqb[:, :, 128:256],
                                    sgb[:, co:co + H, 128:256], MULT)
                    e.tensor_tensor(gtb[:], sqb[:, :, 0:128],
                                    sgb[:, co:co + H, 0:128], MULT)
                    e.tensor_tensor(mgb[:, :, 512:640], sqb[:, :, 256:384],
                                    sgb[:, co:co + H, 256:384], MULT)
                    for d in range(3):
                        e.tensor_tensor(
                            mgb[:, :, 128 + d * 128:256 + d * 128],
                            vgb[:, co:co + H, d * 128:(d + 1) * 128],
                            gtb[:], MULT)
                    rn = _bcast_last(b_rn3[:, t * C + co:t * C + co + H, 0:1], 128)
                    e.tensor_tensor(mgb[:, :, 640:768], mgb[:, :, 512:640],
                                    rn, MULT).then_inc(s_msg, 1)
                if t >= 1:
                    evac(t - 1)
            evac(TILES - 1)

        # ---------------- GPSIMD ----------------
        @block.gpsimd
        def _(e):
            if stage == 1:
                return
            e.load_library(mlp)
            e.wait_ge(s_spw, 16 * NT0)
            if stage == 2:
                for t in range(TILES):
                    e.wait_ge(s_ein, 16 * (t + 1))
                    if t >= 2:
                        e.wait_ge(s_outw, 32 * (t - 1))
                    idx = b_srci[t % 2]
                    T3 = C // 3
                    for th in range(3):
                        ix = idx[:, th * (T3 * 8):(th + 1) * (T3 * 8)]
                        e.dma_gather(b_sg[t % 2][:, th * T3:(th + 1) * T3, :],
                                     d_spass[:], ix, T3 * 128, T3 * 128,
                                     384).then_inc(s_gat, 16)
                        e.dma_gather(b_vg[t % 2][:, th * T3:(th + 1) * T3, :],
                                     d_vtab[:], ix, T3 * 128, T3 * 128,
                                     384).then_inc(s_gat, 16)
                return
            for t in range(TILES):
                e.wait_ge(s_ein, 16 * (3 * t + 1))
                if t >= 2:
                    e.wait_ge(s_msg, 2 * (t - 1))  # sg/vg consumed
                idx = b_srci[t % 2]
                T3 = C // 3
                for th in range(3):
                    ix = idx[:, th * (T3 * 8):(th + 1) * (T3 * 8)]
                    e.dma_gather(b_sg[t % 2][:, th * T3:(th + 1) * T3, :],
                                 d_spass[:], ix, T3 * 128, T3 * 128,
                                 384).then_inc(s_gat, 16)
                    e.dma_gather(b_vg[t % 2][:, th * T3:(th + 1) * T3, :],
                                 d_vtab[:], ix, T3 * 128, T3 * 128,
                                 384).then_inc(s_gat, 16)
                for hh in range(2):
                    ht = 2 * t + hh
                    mgb = b_msg[ht % 2]
                    e.wait_ge(s_msg, ht + 1)
                    co = hh * H
                    for d in (1, 2):
                        rn = _bcast_last(
                            b_rn3[:, t * C + co:t * C + co + H, d:d + 1], 128)
                        ins = e.tensor_tensor(
                            mgb[:, :, 640 + d * 128:768 + d * 128],
                            mgb[:, :, 512:640], rn, MULT)
                    ins.then_inc(s_rhsd, 1)

    nc.compile()
    for t in reversed(ctx):
        t.__exit__(None, None, None)
    return nc


# ---------------------------------------------------------------------------
# Host side
# ---------------------------------------------------------------------------

_CACHE = {}
_LAST_IN_MAPS = None


def _get_program(C):
    if C not in _CACHE:
        _CACHE[C] = _build(C)
    return _CACHE[C]


def kernel(s, v, edges, r_ij, r_ij_normalized, W1, b1, W2, b2, Wrbf, brbf):
    s = np.asarray(s, np.float32)
    v = np.asarray(v, np.float32)
    edges = np.asarray(edges)
    r = np.asarray(r_ij, np.float64)
    rn = np.asarray(r_ij_normalized, np.float32)
    W1 = np.asarray(W1, np.float32)
    W2 = np.asarray(W2, np.float32)
    b1 = np.asarray(b1, np.float32)
    b2 = np.asarray(b2, np.float32)
    Wrbf = np.asarray(Wrbf, np.float32)
    brbf = np.asarray(brbf, np.float32)
    E = edges.shape[0]

    dst = np.asarray(edges[:, 0], np.int64)
    src = np.asarray(edges[:, 1], np.int64)
    order = np.argsort(dst, kind="stable")
    dst_s, src_s = dst[order], src[order]
    r_s, rn_s = r[order], rn[order]

    # per (core, tile) edge ranges
    bounds = np.zeros((NCORES, TILES + 1), np.int64)
    for k in range(NCORES):
        for t in range(TILES + 1):
            node = k * NPC + min(t * 128, NPC)
            bounds[k, t] = np.searchsorted(dst_s, node)
    cnt = bounds[:, 1:] - bounds[:, :-1]
    C = int(np.ceil(cnt.max() / 128.0))
    C = max(6, ((C + 5) // 6) * 6)
    EC = C * 128
    nc = _get_program(C)

    spadT = np.zeros((128, NPADQ), np.float32)
    spadT[:, :N_NODES] = s.T
    vtab = np.zeros((NPAD, 384), _BF)
    vtab[:N_NODES] = v.reshape(N_NODES, 384).astype(_BF)
    Waug = np.zeros((128, 384), np.float32)
    for u in range(3):
        Waug[32 * u:32 * u + 20] = Wrbf
        Waug[32 * u + 20] = brbf

    in_maps = []
    fullcols = np.arange(512)
    for k in range(NCORES):
        srci = np.zeros((TILES, 128, C * 8), np.int16)
        rn3 = np.zeros((128, TILES * C, 3), np.float32)
        ohd = np.zeros((TILES, 128, EC), _BF)
        argA = np.zeros((RQ, 128, EC), np.float32)
        argG = np.zeros((RQ, 128, EC), np.float32)
        ribc = np.zeros((RQ, 128, EC), np.float32)
        base = np.zeros((TILES, 128, 512), np.float32)
        for t in range(TILES):
            lo, hi = bounds[k, t], bounds[k, t + 1]
            n = hi - lo
            e_src = np.zeros(EC, np.int64)
            e_src[:n] = src_s[lo:hi]
            e_dstw = np.full(EC, -1.0, np.float64)
            e_dstw[:n] = dst_s[lo:hi] - (k * NPC + t * 128)
            e_r = np.full(EC, 1.0, np.float64)
            e_r[:n] = r_s[lo:hi]
            e_rn = np.zeros((EC, 3), np.float32)
            e_rn[:n] = rn_s[lo:hi]
            # gather idx: idx i at [16g + i%16, i//16]
            w = e_src.reshape(C * 8, 16).T.astype(np.int16)
            srci[t] = np.tile(w, (8, 1))
            # edge j -> partition j%128, chunk j//128
            pe = np.arange(EC)
            part, ch = pe % 128, pe // 128
            rn3[part, t * C + ch, :] = e_rn
            oh = np.zeros((128, C, 128), _BF)
            valid = e_dstw >= 0
            oh[part[valid], ch[valid], e_dstw[valid].astype(np.int64)] = _BF(1.0)
            ohd[t] = oh.reshape(128, EC)
            # rbf group packing: tile t -> group t//3, partition block 32*(t%3)
            q, u = t // 3, t % 3
            ar = np.zeros((32, EC), np.float64)
            nn = np.arange(1, N_RBF + 1, dtype=np.float64)
            a = nn[:, None] * np.pi * e_r[None, :] / R_CUT
            ar[:20] = np.mod(a + np.pi, 2 * np.pi) - np.pi
            ar[20] = np.pi / 2
            argA[q, 32 * u:32 * u + 32] = ar.astype(np.float32)
            argG[q, 32 * u:32 * u + 32] = (np.pi * e_r / (2 * R_CUT)
                                           + np.pi / 2).astype(np.float32)
            rb = np.zeros((32, EC), np.float32)
            rb[:20] = (1.0 / e_r).astype(np.float32)
            rb[20] = 1.0
            ribc[q, 32 * u:32 * u + 32] = rb
            # base: s|v rows for this tile's nodes
            n0 = k * NPC + t * 128
            nvalid = min(128, NPC - t * 128)
            rows = np.arange(nvalid) + n0
            base[t, :nvalid, 0:128] = s[rows]
            base[t, :nvalid, 128:512] = v[rows].reshape(nvalid, 384)
        in_maps.append({
            "sT": spadT, "W1": W1, "W2": W2, "Waug": Waug,
            "b1c": b1.reshape(128, 1), "b2r": b2.reshape(1, 384),
            "ones": np.ones((1, 128), np.float32),
            "vtab": vtab, "srci": srci, "rn3": rn3, "ohd": ohd,
            "argA": argA, "argG": argG, "ribc": ribc, "base": base,
        })

    global _LAST_IN_MAPS
    _LAST_IN_MAPS = in_maps
    res = run_bass_kernel_spmd(nc, in_maps, core_ids=list(range(NCORES)))
    s_out = np.zeros((N_NODES, EMB), np.float32)
    v_out = np.zeros((N_NODES, 3, EMB), np.float32)
    for k in range(NCORES):
        o = res.results[k]["out"]  # [TILES, 128, 512]
        for t in range(TILES):
            n0 = k * NPC + t * 128
            nvalid = min(128, NPC - t * 128)
            s_out[n0:n0 + nvalid] = o[t, :nvalid, 0:128]
            v_out[n0:n0 + nvalid] = o[t, :nvalid, 128:512].reshape(nvalid, 3, 128)
    return (s_out, v_out)


# revision 18
# speedup vs baseline: 1.0039x; 1.0039x over previous
"""Trainium2 Bass kernel for nn_MessageBlock (GNN message passing).

Sharding: edges sorted by destination node, sharded across 8 cores by
destination range (2500 nodes each) -- every core owns its output slice,
so no inter-core collective is needed.  Per core, edges are bucketed into
20 tiles of 128 destination nodes; each 128-edge chunk's messages are
scatter-added into a PSUM accumulator with a one-hot matmul on the tensor
engine (exact for duplicate destinations).  Source-node features (the
s_pass MLP table and v) are fetched per edge with gpsimd dma_gather.

Device pipeline per core:
  phase 0: s_pass = silu(s@W1+b1)@W2+b2 for all 20096 (padded) nodes,
           written to a bf16 DRAM table.  Transpose-free: W1 is the
           stationary operand so the product comes out feature-major.
  phase 1: per node tile: gather s_pass/v rows by source index, build
           RBF features with ACT sin + an f32r matmul, form messages in
           bf16 on DVE/GPSIMD, scatter via one-hot matmuls into PSUM,
           add the s/v base values, DMA out.
"""
import sys
import dataclasses

sys.path.insert(0, "/opt/trn_rl_repo")

import numpy as np
import ml_dtypes

import concourse.bacc as bacc
import concourse.bass as bass
import concourse.mybir as mybir
from concourse.bass_utils import run_bass_kernel_spmd
from concourse.library_config import mlp

F32 = mybir.dt.float32
F32R = mybir.dt.float32r
BF16 = mybir.dt.bfloat16
I16 = mybir.dt.int16
MULT = mybir.AluOpType.mult
ADD = mybir.AluOpType.add
AF = mybir.ActivationFunctionType

N_NODES = 20000
EMB = 128
N_RBF = 20
R_CUT = 5.0
NCORES = 8
NPC = N_NODES // NCORES          # 2500 nodes per core
TILES = (NPC + 127) // 128       # 20 node tiles per core
NT0 = (N_NODES + 127) // 128     # 157 s_pass tiles
NPAD = NT0 * 128                 # 20096
QUADS0 = (NT0 + 3) // 4          # 40
NPADQ = QUADS0 * 512             # 20480
RQ = (TILES + 2) // 3            # 7 rbf groups (3 tiles each; base 96 illegal)

_BF = ml_dtypes.bfloat16


def _bcast_last(ap, n):
    """View an AP whose last dim is 1 as broadcast to n (stride 0)."""
    new = list(ap.ap)
    new[-1] = (0, n)
    return dataclasses.replace(ap, ap=new)


def _build(C, stage=3):
    """Build the SPMD program; C = edge chunks per node tile (multiple of 6)."""
    EC = C * 128
    H = C // 2
    nc = bacc.Bacc("TRN2")

    d_sT = nc.declare_dram_parameter("sT", [128, NPADQ], F32R, isOutput=False)
    d_W1 = nc.declare_dram_parameter("W1", [128, EMB], F32R, isOutput=False)
    d_W2 = nc.declare_dram_parameter("W2", [128, 3 * EMB], F32R, isOutput=False)
    d_Waug = nc.declare_dram_parameter("Waug", [128, 3 * EMB], F32R, isOutput=False)
    d_b1 = nc.declare_dram_parameter("b1c", [128, 1], F32, isOutput=False)
    d_b2 = nc.declare_dram_parameter("b2r", [1, 3 * EMB], F32R, isOutput=False)
    d_ones = nc.declare_dram_parameter("ones", [1, 128], F32R, isOutput=False)
    d_vtab = nc.declare_dram_parameter("vtab", [NPAD, 3 * EMB], BF16, isOutput=False)
    d_srci = nc.declare_dram_parameter("srci", [TILES, 128, C * 8], I16, isOutput=False)
    d_rn3 = nc.declare_dram_parameter("rn3", [128, TILES * C, 3], F32, isOutput=False)
    d_ohd = nc.declare_dram_parameter("ohd", [TILES, 128, EC], BF16, isOutput=False)
    d_argA = nc.declare_dram_parameter("argA", [RQ, 128, EC], F32, isOutput=False)
    d_argG = nc.declare_dram_parameter("argG", [RQ, 128, EC], F32, isOutput=False)
    d_ribc = nc.declare_dram_parameter("ribc", [RQ, 128, EC], F32, isOutput=False)
    d_base = nc.declare_dram_parameter("base", [TILES, 128, 512], F32, isOutput=False)
    d_out = nc.declare_dram_parameter("out", [TILES, 128, 512], F32, isOutput=True)
    d_spass = nc.dram_tensor("spass", [NPAD, 3 * EMB], BF16)

    ctx = []

    def sb(name, shape, dtype):
        t = nc.sbuf_tensor(name, shape, dtype)
        ctx.append(t)
        return t.__enter__()

    def ps(name, shape):
        t = nc.psum_tensor(name, shape, F32)
        ctx.append(t)
        return t.__enter__()

    def sem(name):
        t = nc.semaphore(name)
        ctx.append(t)
        return t.__enter__()

    # SBUF
    b_W1 = sb("b_W1", [128, EMB], F32R)
    b_W2 = sb("b_W2", [128, 3 * EMB], F32R)
    b_Waug = sb("b_Waug", [128, 3 * EMB], F32R)
    b_b1 = sb("b_b1", [128, 1], F32)
    b_b2 = sb("b_b2", [1, 3 * EMB], F32R)
    b_ones = sb("b_ones", [1, 128], F32R)
    b_rn3 = sb("b_rn3", [128, TILES * C, 3], F32)
    b_sT = [sb(f"b_sT{i}", [128, 512], F32R) for i in range(2)]
    b_hs = [sb(f"b_hs{i}", [128, 512], F32R) for i in range(2)]
    b_sp = [sb(f"b_sp{i}", [128, 3 * EMB], BF16) for i in range(4)]
    b_argA = sb("b_argA", [128, EC], F32)
    b_argG = sb("b_argG", [128, EC], F32)
    b_ribc = sb("b_ribc", [128, EC], F32)
    b_sin = sb("b_sin", [128, EC], F32)
    b_rbf = sb("b_rbf", [128, EC], F32R)
    b_srci = [sb(f"b_srci{i}", [128, C * 8], I16) for i in range(2)]
    b_sg = [sb(f"b_sg{i}", [128, C, 384], BF16) for i in range(2)]
    b_vg = [sb(f"b_vg{i}", [128, C, 384], BF16) for i in range(2)]
    b_oh = [sb(f"b_oh{i}", [128, EC], BF16) for i in range(2)]
    b_base = [sb(f"b_base{i}", [128, 512], F32) for i in range(2)]
    b_out = [sb(f"b_out{i}", [128, 512], F32) for i in range(2)]
    # half-tile buffers; msg cols: [ds 0:128 | vgate 128:512 | rep 512:640 | rhs_d 640:1024]
    b_sq = [sb(f"b_sq{i}", [128, H, 384], BF16) for i in range(3)]
    b_msg = [sb(f"b_msg{i}", [128, H, 1024], BF16) for i in range(2)]
    b_gate = [sb(f"b_gate{i}", [128, H, 128], BF16) for i in range(2)]

    # PSUM (8 banks)
    p_out = [ps(f"p_out{i}", [128, 512]) for i in range(2)]
    p_t = ps("p_t", [128, 3 * 512])
    p_h = ps("p_h", [128, 512])
    p_sp = [ps(f"p_sp{i}", [128, 512]) for i in range(2)]

    s_c0 = sem("s_c0")
    s_sTin = sem("s_sTin")
    s_h = sem("s_h")
    s_silu = sem("s_silu")
    s_mm2 = sem("s_mm2")
    s_evD = sem("s_evD")     # phase-0 evac, even tiles (DVE)
    s_evA = sem("s_evA")     # phase-0 evac, odd tiles (ACT)
    s_spw = sem("s_spw")
    s_rbin = sem("s_rbin")
    s_sin = sem("s_sin")
    s_rbf = sem("s_rbf")
    s_ein = sem("s_ein")
    s_gat = sem("s_gat")
    s_tmm = sem("s_tmm")
    s_sq = sem("s_sq")
    s_msg = sem("s_msg")     # DVE per-half msg ops (before rhs_d d=1,2)
    s_rhsd = sem("s_rhsd")   # gpsimd rhs_d per half
    s_smmh = sem("s_smmh")   # PE scatter per half
    s_acc = sem("s_acc")
    s_outw = sem("s_outw")

    NCONST = 7  # W1 W2 Waug b1 b2 ones rn3

    with nc.Block() as block:
        # ---------------- SYNC: all bulk DMA ----------------
        @block.sync
        def _(e):
            e.dma_start(out=b_W1[:], in_=d_W1[:]).then_inc(s_c0, 16)
            e.dma_start(out=b_W2[:], in_=d_W2[:]).then_inc(s_c0, 16)
            e.dma_start(out=b_Waug[:], in_=d_Waug[:]).then_inc(s_c0, 16)
            e.dma_start(out=b_b1[:], in_=d_b1[:]).then_inc(s_c0, 16)
            e.dma_start(out=b_b2[:], in_=d_b2[:]).then_inc(s_c0, 16)
            e.dma_start(out=b_ones[:], in_=d_ones[:]).then_inc(s_c0, 16)
            e.dma_start(out=b_rn3[:], in_=d_rn3[:]).then_inc(s_c0, 16)
            def spass_store_quad(qq):
                for t in range(4 * qq, min(4 * qq + 4, NT0)):
                    e.wait_ge(s_evD if t % 2 == 0 else s_evA, t // 2 + 1)
                    e.dma_start(out=d_spass[t * 128:(t + 1) * 128, :],
                                in_=b_sp[t % 4][:]).then_inc(s_spw, 16)

            for q in range(QUADS0):
                if q >= 2:
                    e.wait_ge(s_h, q - 1)
                e.dma_start(out=b_sT[q % 2][:],
                            in_=d_sT[:, q * 512:(q + 1) * 512]).then_inc(s_sTin, 16)
                if q >= 2:
                    spass_store_quad(q - 2)
            spass_store_quad(QUADS0 - 2)
            spass_store_quad(QUADS0 - 1)
            if stage == 1:
                # debug: dump spass rows into out
                for t in range(TILES):
                    e.wait_ge(s_spw, 16 * NT0)
                    e.dma_start(out=d_out[t][:].bitcast(BF16)[:, 0:384],
                                in_=d_spass[t * 128:(t + 1) * 128, :]).then_inc(s_outw, 16)
                e.wait_ge(s_outw, 16 * TILES)
                e.wait_ge(s_spw, 16 * NT0)
                return
            if stage == 2:
                # srci loads + dump gathered sg chunk0 per tile
                for t in range(TILES):
                    if t >= 2:
                        e.wait_ge(s_gat, 96 * (t - 1))
                    e.dma_start(out=b_srci[t % 2][:], in_=d_srci[t]).then_inc(s_ein, 16)
                    e.wait_ge(s_gat, 96 * (t + 1))
                    e.dma_start(out=d_out[t][:].bitcast(BF16)[:, 0:768],
                                in_=b_sg[t % 2][:, 0:2, :]).then_inc(s_outw, 16)
                    e.dma_start(out=d_out[t][:].bitcast(BF16)[:, 768:1024][:, 0:256],
                                in_=b_vg[t % 2][:, 0, 0:256]).then_inc(s_outw, 16)
                e.wait_ge(s_outw, 32 * TILES)
                e.wait_ge(s_spw, 16 * NT0)
                return
            for t in range(TILES):
                q = t // 3
                if t % 3 == 0:
                    if q >= 1:
                        e.wait_ge(s_tmm, 3 * q * C)
                    e.dma_start(out=b_argA[:], in_=d_argA[q]).then_inc(s_rbin, 16)
                    e.dma_start(out=b_argG[:], in_=d_argG[q]).then_inc(s_rbin, 16)
                    e.dma_start(out=b_ribc[:], in_=d_ribc[q]).then_inc(s_rbin, 16)
                if t >= 2:
                    e.wait_ge(s_gat, 96 * (t - 1))   # srci consumed by gather t-2
                    e.wait_ge(s_smmh, 2 * (t - 1))   # oh consumed by scatter t-2
                    e.wait_ge(s_acc, t - 1)          # base consumed by evac t-2
                e.dma_start(out=b_srci[t % 2][:], in_=d_srci[t]).then_inc(s_ein, 16)
                e.dma_start(out=b_oh[t % 2][:], in_=d_ohd[t]).then_inc(s_ein, 16)
                e.dma_start(out=b_base[t % 2][:], in_=d_base[t]).then_inc(s_ein, 16)
                if t >= 2:
                    e.wait_ge(s_acc, t - 1)
                    e.dma_start(out=d_out[t - 2],
                                in_=b_out[t % 2][:]).then_inc(s_outw, 16)
            for t in range(TILES - 2, TILES):
                e.wait_ge(s_acc, t + 1)
                e.dma_start(out=d_out[t], in_=b_out[t % 2][:]).then_inc(s_outw, 16)
            e.wait_ge(s_outw, 16 * TILES)
            e.wait_ge(s_spw, 16 * NT0)

        # ---------------- PE ----------------
        @block.tensor
        def _(e):
            e.wait_ge(s_c0, 16 * NCONST)
            w1r = b_W1[:]
            w2r = b_W2[:]
            onesr = b_ones[:]
            b2r = b_b2[:]

            def mm2_quad(q):
                for u in range(4):
                    t = q * 4 + u
                    if t >= NT0:
                        return
                    if t >= 2:
                        ev = s_evD if (t - 2) % 2 == 0 else s_evA
                        e.wait_ge(ev, (t - 2) // 2 + 1)
                    hsr = b_hs[q % 2][:, u * 128:(u + 1) * 128]
                    e.matmul(p_sp[t % 2][:, 0:384], hsr, w2r,
                             start=True, stop=False, skip_group_check=True)
                    e.matmul(p_sp[t % 2][:, 0:384], onesr, b2r,
                             start=False, stop=True,
                             skip_group_check=True).then_inc(s_mm2, 1)

            for q in range(QUADS0):
                e.wait_ge(s_sTin, 16 * (q + 1))
                if q >= 1:
                    e.wait_ge(s_silu, q)      # p_h consumed by silu(q-1)
                e.matmul(p_h[:], w1r, b_sT[q % 2][:],
                         start=True, stop=True, skip_group_check=True).then_inc(s_h, 1)
                if q >= 1:
                    e.wait_ge(s_silu, q)
                    mm2_quad(q - 1)
            e.wait_ge(s_silu, QUADS0)
            mm2_quad(QUADS0 - 1)

            def scatter(t):
                ohb = b_oh[t % 2]
                if t >= 2:
                    e.wait_ge(s_acc, t - 1)   # p_out[t%2] free
                for hh in range(2):
                    ht = 2 * t + hh
                    e.wait_ge(s_msg, ht + 1)
                    e.wait_ge(s_rhsd, ht + 1)
                    mb = b_msg[ht % 2]
                    last = None
                    for c in range(H):
                        cg = hh * H + c
                        lhs = ohb[:, cg * 128:(cg + 1) * 128]
                        e.matmul(p_out[t % 2][:], lhs, mb[:, c, 0:512],
                                 start=(cg == 0), stop=False, skip_group_check=True)
                        last = e.matmul(
                            p_out[t % 2][:, 128:512],
                            lhs, mb[:, c, 640:1024],
                            start=False, stop=(cg == C - 1),
                            skip_group_check=True)
                    last.then_inc(s_smmh, 1)

            if stage < 3:
                return
            # phase 1: t-matmuls for tile t, then scatter for tile t-1
            for t in range(TILES):
                q = t // 3
                e.wait_ge(s_rbf, q + 1)
                e.wait_ge(s_ein, 16 * (3 * t + 2))   # oh(t) loaded (for scatter later)
                for c in range(C):
                    k = t * C + c
                    if k >= 3:
                        e.wait_ge(s_sq, (k - 3) // 3 + 1)  # p_t bank free
                    lhs = b_rbf[32 * (t % 3):32 * (t % 3) + 21,
                                c * 128:(c + 1) * 128]
                    rhs = b_Waug[32 * (t % 3):32 * (t % 3) + 21, :]
                    e.matmul(p_t[:, (c % 3) * 512:(c % 3) * 512 + 384],
                             lhs, rhs, start=True, stop=True,
                             skip_group_check=True).then_inc(s_tmm, 1)
                if t >= 1:
                    scatter(t - 1)
            scatter(TILES - 1)

        # ---------------- ACT ----------------
        @block.scalar
        def _(e):
            e.wait_ge(s_c0, 16 * NCONST)

            def evac_odd(qq):
                for t in range(4 * qq, min(4 * qq + 4, NT0)):
                    if t % 2 != 1:
                        continue
                    e.wait_ge(s_mm2, t + 1)
                    if t >= 4:
                        e.wait_ge(s_spw, 16 * (t - 3))
                    e.activation(b_sp[t % 4][:], p_sp[t % 2][:, 0:384],
                                 AF.Copy).then_inc(s_evA, 1)

            for q in range(QUADS0):
                if q >= 2:
                    evac_odd(q - 2)     # before silu(q): breaks mm2->evac cycle
                e.wait_ge(s_h, q + 1)
                if q >= 2:
                    lim = min(4 * (q - 1), NT0)
                    e.wait_ge(s_mm2, lim)    # b_hs[q%2] consumed
                e.activation(b_hs[q % 2][:], p_h[:], AF.Silu,
                             bias=b_b1[:, 0:1], scale=1.0).then_inc(s_silu, 1)
            evac_odd(QUADS0 - 2)
            evac_odd(QUADS0 - 1)
            if stage < 3:
                return
            # phase 1
            for t in range(TILES):
                q = t // 3
                if t % 3 == 0:
                    e.wait_ge(s_rbin, 48 * (q + 1))
                    e.activation(b_sin[:], b_argA[:], AF.Sin).then_inc(s_sin, 1)
                    e.activation(b_argA[:], b_argG[:], AF.Sin).then_inc(s_sin, 1)
                for bb in range(C // 3):
                    k0 = t * C + bb * 3
                    e.wait_ge(s_tmm, k0 + 3)
                    hh = (bb * 3) // H
                    lc = (bb * 3) % H
                    ht = 2 * t + hh
                    if ht >= 3:
                        e.wait_ge(s_msg, ht - 2)   # b_sq[ht%3] consumed
                    src = dataclasses.replace(
                        p_t[:], ap=[p_t[:].ap[0], (512, 3), (1, 384)])
                    e.activation(b_sq[ht % 3][:, lc:lc + 3, :], src,
                                 AF.Square).then_inc(s_sq, 1)

        # ---------------- DVE ----------------
        @block.vector
        def _(e):
            e.wait_ge(s_c0, 16 * NCONST)
            # phase-0 evac, even tiles
            for t in range(0, NT0, 2):
                e.wait_ge(s_mm2, t + 1)
                if t >= 4:
                    e.wait_ge(s_spw, 16 * (t - 3))
                e.tensor_copy(b_sp[t % 4][:],
                              p_sp[t % 2][:, 0:384]).then_inc(s_evD, 1)

            if stage < 3:
                return

            def evac(t):
                e.wait_ge(s_smmh, 2 * (t + 1))
                e.wait_ge(s_ein, 16 * (3 * t + 3))
                if t >= 2:
                    e.wait_ge(s_outw, 16 * (t - 1))
                e.tensor_tensor(b_out[t % 2][:], p_out[t % 2][:],
                                b_base[t % 2][:], ADD).then_inc(s_acc, 1)

            for t in range(TILES):
                q = t // 3
                if t % 3 == 0:
                    e.wait_ge(s_sin, 2 * (q + 1))
                    e.tensor_tensor(b_argA[:], b_argA[:], b_ribc[:], MULT)
                    e.tensor_tensor(b_rbf[:], b_sin[:], b_argA[:],
                                    MULT).then_inc(s_rbf, 1)
                for hh in range(2):
                    ht = 2 * t + hh
                    e.wait_ge(s_gat, 96 * t + (64 if hh == 0 else 96))
                    sqb = b_sq[ht % 3]
                    mgb = b_msg[ht % 2]
                    gtb = b_gate[ht % 2]
                    sgb = b_sg[t % 2]
                    vgb = b_vg[t % 2]
                    co = hh * H
                    e.wait_ge(s_sq, t * (C // 3) + (hh + 1) * (H // 3))
                    if ht >= 2:
                        e.wait_ge(s_smmh, ht - 1)  # msg/gate buf consumed
                    e.tensor_tensor(mgb[:, :, 0:128], sqb[:, :, 128:256],
                                    sgb[:, co:co + H, 128:256], MULT)
                    e.tensor_tensor(gtb[:], sqb[:, :, 0:128],
                                    sgb[:, co:co + H, 0:128], MULT)
                    e.tensor_tensor(mgb[:, :, 512:640], sqb[:, :, 256:384],
                                    sgb[:, co:co + H, 256:384], MULT)
                    for d in range(3):
                        e.tensor_tensor(
                            mgb[:, :, 128 + d * 128:256 + d * 128],
                            vgb[:, co:co + H, d * 128:(d + 1) * 128],
                            gtb[:], MULT)
                    rn = _bcast_last(b_rn3[:, t * C + co:t * C + co + H, 0:1], 128)
                    e.tensor_tensor(mgb[:, :, 640:768], mgb[:, :, 512:640],
                                    rn, MULT).then_inc(s_msg, 1)
                if t >= 1:
                    evac(t - 1)
            evac(TILES - 1)

        # ---------------- GPSIMD ----------------
        @block.gpsimd
        def _(e):
            if stage == 1:
                return
            e.load_library(mlp)
            e.wait_ge(s_spw, 16 * NT0)
            if stage == 2:
                for t in range(TILES):
                    e.wait_ge(s_ein, 16 * (t + 1))
                    if t >= 2:
                        e.wait_ge(s_outw, 32 * (t - 1))
                    idx = b_srci[t % 2]
                    T3 = C // 3
                    for th in range(3):
                        ix = idx[:, th * (T3 * 8):(th + 1) * (T3 * 8)]
                        e.dma_gather(b_sg[t % 2][:, th * T3:(th + 1) * T3, :],
                                     d_spass[:], ix, T3 * 128, T3 * 128,
                                     384).then_inc(s_gat, 16)
                        e.dma_gather(b_vg[t % 2][:, th * T3:(th + 1) * T3, :],
                                     d_vtab[:], ix, T3 * 128, T3 * 128,
                                     384).then_inc(s_gat, 16)
                return
            for t in range(TILES):
                e.wait_ge(s_ein, 16 * (3 * t + 1))
                if t >= 2:
                    e.wait_ge(s_msg, 2 * (t - 1))  # sg/vg consumed
                idx = b_srci[t % 2]
                T3 = C // 3
                for th in range(3):
                    ix = idx[:, th * (T3 * 8):(th + 1) * (T3 * 8)]
                    e.dma_gather(b_sg[t % 2][:, th * T3:(th + 1) * T3, :],
                                 d_spass[:], ix, T3 * 128, T3 * 128,
                                 384).then_inc(s_gat, 16)
                    e.dma_gather(b_vg[t % 2][:, th * T3:(th + 1) * T3, :],
                                 d_vtab[:], ix, T3 * 128, T3 * 128,
                                 384).then_inc(s_gat, 16)
                for hh in range(2):
                    ht = 2 * t + hh
                    mgb = b_msg[ht % 2]
                    e.wait_ge(s_msg, ht + 1)
                    co = hh * H
                    for d in (1, 2):
                        rn = _bcast_last(
                            b_rn3[:, t * C + co:t * C + co + H, d:d + 1], 128)
                        ins = e.tensor_tensor(
                            mgb[:, :, 640 + d * 128:768 + d * 128],
                            mgb[:, :, 512:640], rn, MULT)
                    ins.then_inc(s_rhsd, 1)

    nc.compile()
    for t in reversed(ctx):
        t.__exit__(None, None, None)
    return nc


# ---------------------------------------------------------------------------
# Host side
# ---------------------------------------------------------------------------

_CACHE = {}
_LAST_IN_MAPS = None


def _get_program(C):
    if C not in _CACHE:
        _CACHE[C] = _build(C)
    return _CACHE[C]


def kernel(s, v, edges, r_ij, r_ij_normalized, W1, b1, W2, b2, Wrbf, brbf):
    s = np.asarray(s, np.float32)
    v = np.asarray(v, np.float32)
    edges = np.asarray(edges)
    r = np.asarray(r_ij, np.float64)
    rn = np.asarray(r_ij_normalized, np.float32)
    W1 = np.asarray(W1, np.float32)
    W2 = np.asarray(W2, np.float32)
    b1 = np.asarray(b1, np.float32)
    b2 = np.asarray(b2, np.float32)
    Wrbf = np.asarray(Wrbf, np.float32)
    brbf = np.asarray(brbf, np.float32)
    E = edges.shape[0]

    dst = np.asarray(edges[:, 0], np.int64)
    src = np.asarray(edges[:, 1], np.int64)
    order = np.argsort(dst, kind="stable")
    dst_s, src_s = dst[order], src[order]
    r_s, rn_s = r[order], rn[order]

    # per (core, tile) edge ranges
    bounds = np.zeros((NCORES, TILES + 1), np.int64)
    for k in range(NCORES):
        for t in range(TILES + 1):
            node = k * NPC + min(t * 128, NPC)
            bounds[k, t] = np.searchsorted(dst_s, node)
    cnt = bounds[:, 1:] - bounds[:, :-1]
    C = int(np.ceil(cnt.max() / 128.0))
    C = max(6, ((C + 5) // 6) * 6)
    EC = C * 128
    nc = _get_program(C)

    spadT = np.zeros((128, NPADQ), np.float32)
    spadT[:, :N_NODES] = s.T
    vtab = np.zeros((NPAD, 384), _BF)
    vtab[:N_NODES] = v.reshape(N_NODES, 384).astype(_BF)
    Waug = np.zeros((128, 384), np.float32)
    for u in range(3):
        Waug[32 * u:32 * u + 20] = Wrbf
        Waug[32 * u + 20] = brbf

    in_maps = []
    fullcols = np.arange(512)
    for k in range(NCORES):
        srci = np.zeros((TILES, 128, C * 8), np.int16)
        rn3 = np.zeros((128, TILES * C, 3), np.float32)
        ohd = np.zeros((TILES, 128, EC), _BF)
        argA = np.zeros((RQ, 128, EC), np.float32)
        argG = np.zeros((RQ, 128, EC), np.float32)
        ribc = np.zeros((RQ, 128, EC), np.float32)
        base = np.zeros((TILES, 128, 512), np.float32)
        for t in range(TILES):
            lo, hi = bounds[k, t], bounds[k, t + 1]
            n = hi - lo
            e_src = np.zeros(EC, np.int64)
            e_src[:n] = src_s[lo:hi]
            e_dstw = np.full(EC, -1.0, np.float64)
            e_dstw[:n] = dst_s[lo:hi] - (k * NPC + t * 128)
            e_r = np.full(EC, 1.0, np.float64)
            e_r[:n] = r_s[lo:hi]
            e_rn = np.zeros((EC, 3), np.float32)
            e_rn[:n] = rn_s[lo:hi]
            # gather idx: idx i at [16g + i%16, i//16]
            w = e_src.reshape(C * 8, 16).T.astype(np.int16)
            srci[t] = np.tile(w, (8, 1))
            # edge j -> partition j%128, chunk j//128
            pe = np.arange(EC)
            part, ch = pe % 128, pe // 128
            rn3[part, t * C + ch, :] = e_rn
            oh = np.zeros((128, C, 128), _BF)
            valid = e_dstw >= 0
            oh[part[valid], ch[valid], e_dstw[valid].astype(np.int64)] = _BF(1.0)
            ohd[t] = oh.reshape(128, EC)
            # rbf group packing: tile t -> group t//3, partition block 32*(t%3)
            q, u = t // 3, t % 3
            ar = np.zeros((32, EC), np.float64)
            nn = np.arange(1, N_RBF + 1, dtype=np.float64)
            a = nn[:, None] * np.pi * e_r[None, :] / R_CUT
            ar[:20] = np.mod(a + np.pi, 2 * np.pi) - np.pi
            ar[20] = np.pi / 2
            argA[q, 32 * u:32 * u + 32] = ar.astype(np.float32)
            argG[q, 32 * u:32 * u + 32] = (np.pi * e_r / (2 * R_CUT)
                                           + np.pi / 2).astype(np.float32)
            rb = np.zeros((32, EC), np.float32)
            rb[:20] = (1.0 / e_r).astype(np.float32)
            rb[20] = 1.0
            ribc[q, 32 * u:32 * u + 32] = rb
            # base: s|v rows for this tile's nodes
            n0 = k * NPC + t * 128
            nvalid = min(128, NPC - t * 128)
            rows = np.arange(nvalid) + n0
            base[t, :nvalid, 0:128] = s[rows]
            base[t, :nvalid, 128:512] = v[rows].reshape(nvalid, 384)
        in_maps.append({
            "sT": spadT, "W1": W1, "W2": W2, "Waug": Waug,
            "b1c": b1.reshape(128, 1), "b2r": b2.reshape(1, 384),
            "ones": np.ones((1, 128), np.float32),
            "vtab": vtab, "srci": srci, "rn3": rn3, "ohd": ohd,
            "argA": argA, "argG": argG, "ribc": ribc, "base": base,
        })

    global _LAST_IN_MAPS
    _LAST_IN_MAPS = in_maps
    res = run_bass_kernel_spmd(nc, in_maps, core_ids=list(range(NCORES)))
    s_out = np.zeros((N_NODES, EMB), np.float32)
    v_out = np.zeros((N_NODES, 3, EMB), np.float32)
    for k in range(NCORES):
        o = res.results[k]["out"]  # [TILES, 128, 512]
        for t in range(TILES):
            n0 = k * NPC + t * 128
            nvalid = min(128, NPC - t * 128)
            s_out[n0:n0 + nvalid] = o[t, :nvalid, 0:128]
            v_out[n0:n0 + nvalid] = o[t, :nvalid, 128:512].reshape(nvalid, 3, 128)
    return (s_out, v_out)
